# revision 42
# baseline (speedup 1.0000x reference)
"""Trainium2 Bass kernel for nn_NestedFormula.

Tree: DEPTH=4, V=4. Level sizes n4=1, n3=5, n2=25, n1=125, n0=125.
  f1[n] = sum_v lam1[n,v] * x_v^pow1[n,v] + lam0[n]
  fd[n] = sum_v lamd[n,v] * x_v^powd[n,v] * f_{d-1}[5n+v] + f_{d-1}[5n+4]
  out   = f4[0]                          (per batch element)

Strategy (pure data parallel over batch, 8 cores x 16384):
  - ACT (scalar engine) is the hard floor: ~630 exps per batch element at
    1 elem/cycle/lane; cost scales with free-dim length only, so every
    activation runs 128 partitions wide with maximal free dim.
  - x^p = exp(p * ln x): one packed Ln [128,512]; per-level Exp calls with
    per-partition scale vectors. Level-1 passthrough/lam0 handled by
    exp(0)=1 rows feeding weight columns of block-diagonal G matmuls.
  - ln(x) bounced to DRAM once (fp16), broadcast-read into replicated
    layouts with step-0 partition APs.
  - All e-tiles, replicas and G weights are fp16 (halves SBUF + DMA and
    keeps 4 extra mantissa bits vs bf16); PSUM accumulates fp32.
  - f2 drain fused into the level-3 multiply for late chunks (reads PSUM
    directly); early chunks drain f2 to SBUF so PSUM slots recycle while
    the l3x replica DMA chain is still in flight.
  - Level-4 exp runs packed [80,1024] (ACT cost is per free-dim element)
    and unpacks via a DRAM bounce.
  - Chunk 3 runs quarter-granular exps with levels 1-4 software-pipelined
    in skewed wavefronts (512-col PSUM pieces, psA/psB x4 bank rotation)
    so the post-exp serial tail is short; final output copies ride the
    then-idle scalar engine.
  - One preloaded ACT table set (natural_log_exp) serves both Ln and Exp.
"""
import numpy as np

import concourse.bacc as bacc
import concourse.mybir as mybir
from concourse.tile import TileContext

DEPTH = 4
V = 4
B = 131072
M_CORES = 8
BS = B // M_CORES          # 16384 per core
CHUNK = 4096
NCH = BS // CHUNK          # 4
HALF = 2048
MMN = 512                  # matmul free dim (one PSUM bank)

F32 = mybir.dt.float32
F32R = mybir.dt.float32r
F16 = mybir.dt.float16

N1, N2, N3, N4 = 125, 25, 5, 1
NT1 = 4                    # level-1 j-tiles of 128


def _sigma1(m):
    # psum1 row m -> level-1 node index
    if m < 100:
        return 5 * (m // 4) + (m % 4)
    return 5 * (m - 100) + 4


def _tau2(m):
    # psum2 row m -> level-2 node index
    if m < 20:
        return 5 * (m // 4) + (m % 4)
    return 5 * (m - 20) + 4


def build_constants(lam0, lam1, pow1, lam2, pow2, lam3, pow3, lam4, pow4):
    c = {}
    # ---- level 1: 4 j-tiles of K=128, M=128 (125 used cols) ----
    sc1 = np.zeros((128, NT1), np.float32)
    g1 = np.zeros((NT1, 128, 128), np.float32)
    for n in range(N1):
        for v in range(V):
            j = 4 * n + v
            t, r = divmod(j, 128)
            sc1[r, t] = pow1[n, v]
    for m in range(125):
        n = _sigma1(m)
        for v in range(V):
            j = 4 * n + v
            t, r = divmod(j, 128)
            g1[t, r, m] = lam1[n, v]
        g1[3, 116, m] = lam0[n]          # ones-row (sc1[116,3]=0 -> exp=1)
    c["sc1"] = sc1
    c["g1"] = np.ascontiguousarray(g1.transpose(1, 0, 2).reshape(128, NT1 * 128))

    # ---- level 2: K=128 (100 exp rows + 25 passthrough), M=32 (25 used) ----
    sc2 = np.zeros((128, 1), np.float32)
    g2 = np.zeros((128, 32), np.float32)
    for n in range(N2):
        for v in range(V):
            sc2[4 * n + v, 0] = pow2[n, v]
    for m in range(25):
        n2t = _tau2(m)
        for v in range(V):
            g2[4 * n2t + v, m] = lam2[n2t, v]
        g2[100 + n2t, m] = 1.0           # + f1[5*n2t+4] passthrough
    c["sc2"] = sc2
    c["g2"] = g2

    # ---- level 3 (chunk-packed): rows 32c+m2, cols 5c+u ----
    sc3 = np.zeros((128, 1), np.float32)
    g3 = np.zeros((128, 32), np.float32)
    for cc in range(NCH):
        for m2 in range(25):
            r = 32 * cc + m2
            if m2 < 20:
                n3, v3 = divmod(m2, 4)
                sc3[r, 0] = pow3[n3, v3]
                g3[r, 5 * cc + n3] = lam3[n3, v3]
            else:
                g3[r, 5 * cc + (m2 - 20)] = 1.0   # + f2[5*n3+4]
    c["sc3"] = sc3
    c["g3"] = g3

    # ---- level 4 (chunk-packed): rows 5c+u (20 rows), cols c ----
    sc4 = np.zeros((20, 1), np.float32)
    g4 = np.zeros((20, NCH), np.float32)
    for cc in range(NCH):
        for u in range(4):
            sc4[5 * cc + u, 0] = pow4[0, u]
            g4[5 * cc + u, cc] = lam4[0, u]
        g4[5 * cc + 4, cc] = 1.0                  # + f3[4]
    c["sc4"] = sc4
    c["g4"] = g4
    # pack: one scales tensor + one weights tensor (fewer DMA triggers)
    scs = np.zeros((128, 7), np.float32)
    scs[:, 0:4] = c["sc1"]
    scs[:, 4:5] = c["sc2"]
    scs[:, 5:6] = c["sc3"]
    scs[0:80, 6:7] = np.repeat(c["sc4"], 4, axis=0)
    gs = np.zeros((128, 580), np.float32)
    gs[:, 0:512] = c["g1"]
    gs[:, 512:544] = c["g2"]
    gs[:, 544:576] = c["g3"]
    gs[0:20, 576:580] = c["g4"]
    return {"scs": scs, "gs": gs.astype(np.float16)}


def build_bass():
    nc = bacc.Bacc()
    xt = nc.dram_tensor("xt", (V, BS), F32, kind="ExternalInput")
    scs = nc.dram_tensor("scs", (128, 7), F32, kind="ExternalInput")
    gs = nc.dram_tensor("gs", (128, 580), F16, kind="ExternalInput")
    y = nc.dram_tensor("y", (BS,), F32, kind="ExternalOutput")

    EXP = mybir.ActivationFunctionType.Exp
    LN = mybir.ActivationFunctionType.Ln

    with TileContext(nc) as tc:
        with tc.tile_pool(name="const", bufs=1) as cpool, \
             tc.tile_pool(name="dram", bufs=1, space="DRAM") as dpool, \
             tc.tile_pool(name="big", bufs=1) as bpool, \
             tc.tile_pool(name="psum", bufs=2, space="PSUM") as ppool:

            # ---------- x in first: it gates ln and the whole exp chain ---
            # row 4g+v, col i  <->  x[v, 512g+i]
            xc = cpool.tile([128, 512], F32, tag="xc")
            nc.sync.dma_start(
                out=xc[:], in_=xt[:, :].rearrange("v (g i) -> g v i", i=512))

            # ---------- constants into SBUF (packed, 2 triggers) ----------
            sct = cpool.tile([128, 7], F32, tag="sct")
            nc.sync.dma_start(out=sct[:], in_=scs[:, :])
            sct1, sct2, sct3 = sct[:, 0:4], sct[:, 4:5], sct[:, 5:6]
            sct4 = sct[0:80, 6:7]

            # preload the one ACT table set holding BOTH ln and exp, so the
            # compiler's per-function pass doesn't emit two separate loads
            nc.scalar.add_instruction(mybir.InstLoadActFuncSet(
                name=nc.get_next_instruction_name(), act_func_set_id=6,
                ins=[], outs=[]))

            # ---------- ln(x): one packed [128,512] call, fp16 out ---------
            lc = cpool.tile([128, 512], F16, tag="lc")
            nc.scalar.activation(lc[:], xc[:], LN)
            # bounce to DRAM for broadcast reads; triggers from the ACT ring
            # so they issue the moment ln retires (no SP-ring dispatch wait).
            # Chunk 0's columns go in a separate small first DMA so lrep0's
            # wait is satisfied ~1us earlier (it gates the whole exp stream).
            ld = dpool.tile([V, BS], F16, tag="ld")
            nc.scalar.dma_start(
                out=ld[:, 0:CHUNK].rearrange("v (g i) -> g v i", i=512),
                in_=lc[0:32, :])
            nc.scalar.dma_start(
                out=ld[:, CHUNK:BS].rearrange("v (g i) -> g v i", i=512),
                in_=lc[32:128, :])

            # ---------- lrep chunks 0/1 first: they gate the first exps ----
            # (HWDGE ring is FIFO per issuing engine — queue order matters)
            lreps = [None] * NCH

            def load_lrep(cc, eng=None):
                lrep = bpool.tile([128, CHUNK], F16, tag="lrep", bufs=4,
                                  name=f"lrep{cc}")
                (eng or nc.sync).dma_start(
                    out=lrep[:],
                    in_=ld[:, cc * CHUNK:(cc + 1) * CHUNK].unsqueeze(0)
                        .broadcast_to([32, V, CHUNK]))
                lreps[cc] = lrep

            load_lrep(0)
            load_lrep(1)

            # ---------- phase-B exp inputs (replicas) ----------
            # ld8: ln(x) rows replicated 8x in DRAM (row 4q+v = lnx[v])
            ld8 = dpool.tile([32, BS], F16, tag="ld8")
            nc.sync.dma_start(
                out=ld8[:, :],
                in_=ld[:, :].unsqueeze(0).broadcast_to([8, V, BS]))
            # l3x rows 32c+m2 (m2=4q+v3 for q<5; rows 20.. passthrough pad)
            l3x = bpool.tile([128, CHUNK], F16, tag="l3x", bufs=1)
            nc.sync.dma_start(
                out=l3x[:, :],
                in_=ld8[:, :].rearrange("m (c f) -> c m f", f=CHUNK))

            # G weights (first matmul needs them ~20us in)
            gt = cpool.tile([128, 580], F16, tag="gt")
            nc.sync.dma_start(out=gt[:], in_=gs[:, :])
            g1t = gt[:, 0:512]
            g2t = gt[:, 512:544]
            g3t = gt[:, 544:576]
            g4t = gt[0:20, 576:580]

            load_lrep(2)

            # l4x rows m=5c+u (u<4 real, u=4 passthrough w/ scale 0; ld8
            # row 4 is lnx[0], a valid finite filler for the passthrough).
            # The exp runs PACKED [80,1024] (row 4m+b; ACT cost is per
            # free-dim element, so 4x cheaper than [20,4096]), then unpacks.
            ld20b = dpool.tile([20, CHUNK], F16, tag="ld20b")
            nc.sync.dma_start(
                out=ld20b[:, :],
                in_=ld8[0:5, :].rearrange("u (c f) -> c u f", f=CHUNK))
            l4x = bpool.tile([20, CHUNK], F16, tag="l4x", bufs=1)
            e4pk = bpool.tile([80, 1024], F16, tag="e4pk", bufs=1)
            e4d = dpool.tile([80, 1024], F16, tag="e4d")
            nc.sync.dma_start(
                out=e4pk[:, :],
                in_=ld20b[:, :].rearrange("m (b j) -> (m b) j", j=1024))

            load_lrep(3)

            e2s = [None] * NCH
            PW = 512                   # pipeline piece width (1 PSUM bank)

            # ---------- pipeline stages for one column piece ----------
            def st_ps1(cc, pc, e1s, w=PW):
                ps1 = ppool.tile([128, PW], F32, tag="psA", bufs=4,
                                 name="ps1")[:, 0:w]
                for s in range((w + MMN - 1) // MMN):
                    scol = s * MMN
                    sw = min(MMN, w - scol)
                    for t in range(NT1):
                        nc.tensor.matmul(
                            ps1[:, scol:scol + sw],
                            g1t[:, 128 * t:128 * (t + 1)],
                            e1s[t][:, pc + scol:pc + scol + sw],
                            start=(t == 0), stop=(t == NT1 - 1))
                return ps1

            def st_x2(cc, pc, ps1, w=PW):
                e2 = e2s[cc]
                nc.vector.tensor_mul(
                    e2[:, pc:pc + w], e2[:, pc:pc + w], ps1[:])
                ps2 = ppool.tile([32, PW], F32, tag="psB", bufs=4,
                                 name="ps2")[:, 0:w]
                for s in range((w + MMN - 1) // MMN):
                    scol = s * MMN
                    sw = min(MMN, w - scol)
                    nc.tensor.matmul(
                        ps2[:, scol:scol + sw], g2t[:],
                        e2[:, pc + scol:pc + scol + sw],
                        start=True, stop=True)
                return ps2

            f2e = bpool.tile([64, CHUNK], F16, tag="f2e", bufs=1)

            def st_x3(cc, pc, ps2, w=PW):
                # X3 = E3 * f2
                if cc < 2:
                    # early chunks: the l3x exp may not have landed yet
                    # (its DMA chain is long); drain f2 to SBUF so ps2 slots
                    # recycle immediately, and multiply later (see cc==2)
                    nc.vector.tensor_copy(
                        f2e[32 * cc:32 * cc + 32, pc:pc + w], ps2[:])
                else:
                    # fused drain of ps2 (in place into l3x rows)
                    nc.vector.tensor_mul(
                        l3x[32 * cc:32 * cc + 32, pc:pc + w],
                        l3x[32 * cc:32 * cc + 32, pc:pc + w], ps2[:])

            def st_ps3(pc, w=PW):
                ps3 = ppool.tile([32, PW], F32, tag="psA", bufs=4,
                                 name="ps3")[:, 0:w]
                for s in range((w + MMN - 1) // MMN):
                    scol = s * MMN
                    sw = min(MMN, w - scol)
                    nc.tensor.matmul(
                        ps3[:, scol:scol + sw], g3t[:],
                        l3x[:, pc + scol:pc + scol + sw],
                        start=True, stop=True)
                return ps3

            def st_x4(pc, ps3, w=PW):
                nc.vector.tensor_mul(l4x[:, pc:pc + w], l4x[:, pc:pc + w],
                                     ps3[0:20, :])
                ps4 = ppool.tile([NCH, PW], F32, tag="psB", bufs=4,
                                 name="ps4")[:, 0:w]
                for s in range((w + MMN - 1) // MMN):
                    scol = s * MMN
                    sw = min(MMN, w - scol)
                    nc.tensor.matmul(
                        ps4[:, scol:scol + sw], g4t[:],
                        l4x[:, pc + scol:pc + scol + sw],
                        start=True, stop=True)
                return ps4

            def st_out(pc, ps4, tail, w=PW):
                if tail:
                    nc.scalar.copy(outsb[:, pc:pc + w], ps4[:])  # ACT idle
                else:
                    nc.vector.tensor_copy(outsb[:, pc:pc + w], ps4[:])

            def phase12(cc, pc, e1s):
                """levels 1+2 for chunk cc, columns [pc, pc+PW)."""
                ps1 = st_ps1(cc, pc, e1s)
                ps2 = st_x2(cc, pc, ps1)
                st_x3(cc, pc, ps2)

            def tail_pipeline(pcs, e1s, cc):
                """Chunk-3 pieces (pc, w) pairs, DVE ops emitted in skewed
                wavefronts so the strict-FIFO vector queue never stalls at
                its head."""
                n = len(pcs)
                ps1s = [st_ps1(cc, pc, e1s, w) for pc, w in pcs]
                ps2 = {}
                ps3 = {}
                ps4 = {}
                for d in range(n + 3):
                    for i, (pc, w) in reversed(list(enumerate(pcs))):
                        s = d - i
                        if s == 0:
                            ps2[i] = st_x2(cc, pc, ps1s[i], w)
                        elif s == 1:
                            st_x3(cc, pc, ps2[i], w)
                            ps3[i] = st_ps3(pc, w)
                        elif s == 2:
                            ps4[i] = st_x4(pc, ps3[i], w)
                        elif s == 3:
                            st_out(pc, ps4[i], tail=(pc >= 3072), w=w)

            outsb = bpool.tile([NCH, CHUNK], F32, tag="outsb", bufs=1)

            # ---------- per-chunk: exps then levels 1+2 ----------
            # chunks 0-2: whole-chunk exp calls; chunk 3: quarter-chunk calls
            # with phase 3/4 pieces interleaved so the post-exp tail is short
            NP = CHUNK // PW
            for cc in range(NCH):
                if cc == 2:
                    # deferred l3 mul for chunks 0/1 (fast fp16 SBUF mul)
                    nc.vector.tensor_mul(l3x[0:64, :], l3x[0:64, :],
                                         f2e[:, :])
                last = cc == NCH - 1
                e1s = [None] * NT1
                splits = [(q * 1024, 1024) for q in range(4)] if last \
                    else [(0, CHUNK)]
                for hh, (hc, w) in enumerate(splits):
                    for t in range(NT1):
                        if hh == 0:
                            e1s[t] = bpool.tile([128, CHUNK], F16, tag="e1",
                                                bufs=8, name=f"e1_{cc}_{t}")
                        nc.scalar.activation(
                            e1s[t][:, hc:hc + w], lreps[cc][:, hc:hc + w],
                            EXP, scale=sct1[:, t:t + 1])
                    if cc == 0 and hh == 0:
                        # hoist the small phase-B exps right behind e1 chunk 0
                        nc.scalar.activation(l3x[:], l3x[:], EXP,
                                             scale=sct3[:, 0:1])
                        nc.scalar.activation(e4pk[:], e4pk[:], EXP,
                                             scale=sct4[:, 0:1])
                        # unpack [80,1024] -> [20,4096] via DRAM bounce
                        # (partition-split SBUF source APs are unsupported)
                        nc.sync.dma_start(out=e4d[:, :], in_=e4pk[:, :])
                        nc.sync.dma_start(
                            out=l4x[:, :],
                            in_=e4d[:, :].rearrange("(m b) j -> m b j", b=4))
                    if hh == 0:
                        e2 = bpool.tile([128, CHUNK], F16, tag="e2", bufs=3,
                                        name=f"e2_{cc}")
                        e2s[cc] = e2
                    nc.scalar.activation(e2s[cc][:, hc:hc + w],
                                         lreps[cc][:, hc:hc + w], EXP,
                                         scale=sct2[:, 0:1])
                    if last:
                        pieces = [(p * PW, PW) for p in
                                  range(hc // PW, (hc + w) // PW)]
                        tail_pipeline(pieces, e1s, cc)
                    else:
                        for p in range(hc // PW, (hc + w) // PW):
                            phase12(cc, p * PW, e1s)

            nc.sync.dma_start(
                out=y[:].rearrange("(c i) -> c i", i=CHUNK), in_=outsb[:])

    nc.compile()
    return nc


def kernel(x, lam0, lam1, pow1, lam2, pow2, lam3, pow3, lam4, pow4):
    x = np.asarray(x, np.float32)
    consts = build_constants(
        np.asarray(lam0, np.float32), np.asarray(lam1, np.float32),
        np.asarray(pow1, np.float32), np.asarray(lam2, np.float32),
        np.asarray(pow2, np.float32), np.asarray(lam3, np.float32),
        np.asarray(pow3, np.float32), np.asarray(lam4, np.float32),
        np.asarray(pow4, np.float32))

    nc = build_bass()

    in_maps = []
    for k in range(M_CORES):
        shard = x[k * BS:(k + 1) * BS, :]
        m = {"xt": np.ascontiguousarray(shard.T)}
        m.update(consts)
        in_maps.append(m)

    from concourse.bass_utils import run_bass_kernel_spmd
    res = run_bass_kernel_spmd(nc, in_maps, list(range(M_CORES)))
    out = np.concatenate([res.results[k]["y"] for k in range(M_CORES)])
    return out[:, None].astype(np.float32)


if __name__ == "__main__":
    import reference
    inputs = {k: np.asarray(v) for k, v in reference.setup_inputs().items()}
    got = kernel(**inputs)
    exp = np.asarray(reference.reference(**inputs))
    err = np.abs(got - exp).max() / (np.abs(exp).max() + 1e-30)
    print("shape", got.shape, "relerr", err)


# revision 43
# speedup vs baseline: 1.0216x; 1.0216x over previous
"""Trainium2 Bass kernel for nn_NestedFormula.

Tree: DEPTH=4, V=4. Level sizes n4=1, n3=5, n2=25, n1=125, n0=125.
  f1[n] = sum_v lam1[n,v] * x_v^pow1[n,v] + lam0[n]
  fd[n] = sum_v lamd[n,v] * x_v^powd[n,v] * f_{d-1}[5n+v] + f_{d-1}[5n+4]
  out   = f4[0]                          (per batch element)

Strategy (pure data parallel over batch, 8 cores x 16384):
  - ACT (scalar engine) is the hard floor: ~630 exps per batch element at
    1 elem/cycle/lane; cost scales with free-dim length only, so every
    activation runs 128 partitions wide with maximal free dim.
  - x^p = exp(p * ln x): one packed Ln [128,512]; per-level Exp calls with
    per-partition scale vectors. Level-1 passthrough/lam0 handled by
    exp(0)=1 rows feeding weight columns of block-diagonal G matmuls.
  - ln(x) bounced to DRAM once (fp16), broadcast-read into replicated
    layouts with step-0 partition APs.
  - All e-tiles, replicas and G weights are fp16 (halves SBUF + DMA and
    keeps 4 extra mantissa bits vs bf16); PSUM accumulates fp32.
  - f2 drain fused into the level-3 multiply for late chunks (reads PSUM
    directly); early chunks drain f2 to SBUF so PSUM slots recycle while
    the l3x replica DMA chain is still in flight.
  - Level-4 exp runs packed [80,1024] (ACT cost is per free-dim element)
    and unpacks via a DRAM bounce.
  - Chunk 3 runs quarter-granular exps with levels 1-4 software-pipelined
    in skewed wavefronts (512-col PSUM pieces, psA/psB x4 bank rotation)
    so the post-exp serial tail is short; final output copies ride the
    then-idle scalar engine.
  - One preloaded ACT table set (natural_log_exp) serves both Ln and Exp.
"""
import numpy as np

import concourse.bacc as bacc
import concourse.mybir as mybir
from concourse.tile import TileContext

DEPTH = 4
V = 4
B = 131072
M_CORES = 8
BS = B // M_CORES          # 16384 per core
CHUNK = 4096
NCH = BS // CHUNK          # 4
HALF = 2048
MMN = 512                  # matmul free dim (one PSUM bank)

F32 = mybir.dt.float32
F32R = mybir.dt.float32r
F16 = mybir.dt.float16

N1, N2, N3, N4 = 125, 25, 5, 1
NT1 = 4                    # level-1 j-tiles of 128


def _sigma1(m):
    # psum1 row m -> level-1 node index
    if m < 100:
        return 5 * (m // 4) + (m % 4)
    return 5 * (m - 100) + 4


def _tau2(m):
    # psum2 row m -> level-2 node index
    if m < 20:
        return 5 * (m // 4) + (m % 4)
    return 5 * (m - 20) + 4


def build_constants(lam0, lam1, pow1, lam2, pow2, lam3, pow3, lam4, pow4):
    c = {}
    # ---- level 1: 4 j-tiles of K=128, M=128 (125 used cols) ----
    sc1 = np.zeros((128, NT1), np.float32)
    g1 = np.zeros((NT1, 128, 128), np.float32)
    for n in range(N1):
        for v in range(V):
            j = 4 * n + v
            t, r = divmod(j, 128)
            sc1[r, t] = pow1[n, v]
    for m in range(125):
        n = _sigma1(m)
        for v in range(V):
            j = 4 * n + v
            t, r = divmod(j, 128)
            g1[t, r, m] = lam1[n, v]
        g1[3, 116, m] = lam0[n]          # ones-row (sc1[116,3]=0 -> exp=1)
    c["sc1"] = sc1
    c["g1"] = np.ascontiguousarray(g1.transpose(1, 0, 2).reshape(128, NT1 * 128))

    # ---- level 2: K=128 (100 exp rows + 25 passthrough), M=32 (25 used) ----
    sc2 = np.zeros((128, 1), np.float32)
    g2 = np.zeros((128, 32), np.float32)
    for n in range(N2):
        for v in range(V):
            sc2[4 * n + v, 0] = pow2[n, v]
    for m in range(25):
        n2t = _tau2(m)
        for v in range(V):
            g2[4 * n2t + v, m] = lam2[n2t, v]
        g2[100 + n2t, m] = 1.0           # + f1[5*n2t+4] passthrough
    c["sc2"] = sc2
    c["g2"] = g2

    # ---- level 3 (chunk-packed): rows 32c+m2, cols 5c+u ----
    sc3 = np.zeros((128, 1), np.float32)
    g3 = np.zeros((128, 32), np.float32)
    for cc in range(NCH):
        for m2 in range(25):
            r = 32 * cc + m2
            if m2 < 20:
                n3, v3 = divmod(m2, 4)
                sc3[r, 0] = pow3[n3, v3]
                g3[r, 5 * cc + n3] = lam3[n3, v3]
            else:
                g3[r, 5 * cc + (m2 - 20)] = 1.0   # + f2[5*n3+4]
    c["sc3"] = sc3
    c["g3"] = g3

    # ---- level 4 (chunk-packed): rows 5c+u (20 rows), cols c ----
    sc4 = np.zeros((20, 1), np.float32)
    g4 = np.zeros((20, NCH), np.float32)
    for cc in range(NCH):
        for u in range(4):
            sc4[5 * cc + u, 0] = pow4[0, u]
            g4[5 * cc + u, cc] = lam4[0, u]
        g4[5 * cc + 4, cc] = 1.0                  # + f3[4]
    c["sc4"] = sc4
    c["g4"] = g4
    # pack: one scales tensor + one weights tensor (fewer DMA triggers)
    scs = np.zeros((128, 7), np.float32)
    scs[:, 0:4] = c["sc1"]
    scs[:, 4:5] = c["sc2"]
    scs[:, 5:6] = c["sc3"]
    scs[0:80, 6:7] = np.repeat(c["sc4"], 4, axis=0)
    gs = np.zeros((128, 580), np.float32)
    gs[:, 0:512] = c["g1"]
    gs[:, 512:544] = c["g2"]
    gs[:, 544:576] = c["g3"]
    gs[0:20, 576:580] = c["g4"]
    return {"scs": scs, "gs": gs.astype(np.float16)}


def build_bass():
    nc = bacc.Bacc()
    xt = nc.dram_tensor("xt", (V, BS), F32, kind="ExternalInput")
    scs = nc.dram_tensor("scs", (128, 7), F32, kind="ExternalInput")
    gs = nc.dram_tensor("gs", (128, 580), F16, kind="ExternalInput")
    y = nc.dram_tensor("y", (BS,), F32, kind="ExternalOutput")

    EXP = mybir.ActivationFunctionType.Exp
    LN = mybir.ActivationFunctionType.Ln

    with TileContext(nc) as tc:
        with tc.tile_pool(name="const", bufs=1) as cpool, \
             tc.tile_pool(name="dram", bufs=1, space="DRAM") as dpool, \
             tc.tile_pool(name="big", bufs=1) as bpool, \
             tc.tile_pool(name="psum", bufs=2, space="PSUM") as ppool:

            # ---------- x in first: it gates ln and the whole exp chain ---
            # row 4g+v, col i  <->  x[v, 512g+i]
            xc = cpool.tile([128, 512], F32, tag="xc")
            nc.sync.dma_start(
                out=xc[:], in_=xt[:, :].rearrange("v (g i) -> g v i", i=512))

            # ---------- constants into SBUF (packed, 2 triggers) ----------
            sct = cpool.tile([128, 7], F32, tag="sct")
            nc.sync.dma_start(out=sct[:], in_=scs[:, :])
            sct1, sct2, sct3 = sct[:, 0:4], sct[:, 4:5], sct[:, 5:6]
            sct4 = sct[0:80, 6:7]

            # preload the one ACT table set holding BOTH ln and exp, so the
            # compiler's per-function pass doesn't emit two separate loads
            nc.scalar.add_instruction(mybir.InstLoadActFuncSet(
                name=nc.get_next_instruction_name(), act_func_set_id=6,
                ins=[], outs=[]))

            # ---------- ln(x): one packed [128,512] call, fp16 out ---------
            lc = cpool.tile([128, 512], F16, tag="lc")
            nc.scalar.activation(lc[:], xc[:], LN)
            # bounce to DRAM for broadcast reads; trigger from the ACT ring
            # so it issues the moment ln retires (no SP-ring dispatch wait)
            ld = dpool.tile([V, BS], F16, tag="ld")
            nc.scalar.dma_start(
                out=ld[:, :].rearrange("v (g i) -> g v i", i=512), in_=lc[:])

            # ---------- lrep chunks 0/1 first: they gate the first exps ----
            # (HWDGE ring is FIFO per issuing engine — queue order matters)
            lreps = [None] * NCH

            def load_lrep(cc, eng=None):
                lrep = bpool.tile([128, CHUNK], F16, tag="lrep", bufs=4,
                                  name=f"lrep{cc}")
                (eng or nc.sync).dma_start(
                    out=lrep[:],
                    in_=ld[:, cc * CHUNK:(cc + 1) * CHUNK].unsqueeze(0)
                        .broadcast_to([32, V, CHUNK]))
                lreps[cc] = lrep

            load_lrep(0)
            load_lrep(1)

            # ---------- phase-B exp inputs (replicas) ----------
            # ld8: ln(x) rows replicated 8x in DRAM (row 4q+v = lnx[v])
            ld8 = dpool.tile([32, BS], F16, tag="ld8")
            nc.sync.dma_start(
                out=ld8[:, :],
                in_=ld[:, :].unsqueeze(0).broadcast_to([8, V, BS]))
            # l3x rows 32c+m2 (m2=4q+v3 for q<5; rows 20.. passthrough pad)
            l3x = bpool.tile([128, CHUNK], F16, tag="l3x", bufs=1)
            nc.sync.dma_start(
                out=l3x[:, :],
                in_=ld8[:, :].rearrange("m (c f) -> c m f", f=CHUNK))

            # G weights (first matmul needs them ~20us in)
            gt = cpool.tile([128, 580], F16, tag="gt")
            nc.sync.dma_start(out=gt[:], in_=gs[:, :])
            g1t = gt[:, 0:512]
            g2t = gt[:, 512:544]
            g3t = gt[:, 544:576]
            g4t = gt[0:20, 576:580]

            load_lrep(2)

            # l4x rows m=5c+u (u<4 real, u=4 passthrough w/ scale 0; ld8
            # row 4 is lnx[0], a valid finite filler for the passthrough).
            # The exp runs PACKED [80,1024] (row 4m+b; ACT cost is per
            # free-dim element, so 4x cheaper than [20,4096]), then unpacks.
            ld20b = dpool.tile([20, CHUNK], F16, tag="ld20b")
            nc.sync.dma_start(
                out=ld20b[:, :],
                in_=ld8[0:5, :].rearrange("u (c f) -> c u f", f=CHUNK))
            l4x = bpool.tile([20, CHUNK], F16, tag="l4x", bufs=1)
            e4pk = bpool.tile([80, 1024], F16, tag="e4pk", bufs=1)
            e4d = dpool.tile([80, 1024], F16, tag="e4d")
            nc.sync.dma_start(
                out=e4pk[:, :],
                in_=ld20b[:, :].rearrange("m (b j) -> (m b) j", j=1024))

            load_lrep(3)

            e2s = [None] * NCH
            PW = 512                   # pipeline piece width (1 PSUM bank)

            # ---------- pipeline stages for one column piece ----------
            def st_ps1(cc, pc, e1s, w=PW):
                ps1 = ppool.tile([128, PW], F32, tag="psA", bufs=4,
                                 name="ps1")[:, 0:w]
                for s in range((w + MMN - 1) // MMN):
                    scol = s * MMN
                    sw = min(MMN, w - scol)
                    for t in range(NT1):
                        nc.tensor.matmul(
                            ps1[:, scol:scol + sw],
                            g1t[:, 128 * t:128 * (t + 1)],
                            e1s[t][:, pc + scol:pc + scol + sw],
                            start=(t == 0), stop=(t == NT1 - 1))
                return ps1

            def st_x2(cc, pc, ps1, w=PW):
                e2 = e2s[cc]
                nc.vector.tensor_mul(
                    e2[:, pc:pc + w], e2[:, pc:pc + w], ps1[:])
                ps2 = ppool.tile([32, PW], F32, tag="psB", bufs=4,
                                 name="ps2")[:, 0:w]
                for s in range((w + MMN - 1) // MMN):
                    scol = s * MMN
                    sw = min(MMN, w - scol)
                    nc.tensor.matmul(
                        ps2[:, scol:scol + sw], g2t[:],
                        e2[:, pc + scol:pc + scol + sw],
                        start=True, stop=True)
                return ps2

            f2e = bpool.tile([64, CHUNK], F16, tag="f2e", bufs=1)

            def st_x3(cc, pc, ps2, w=PW):
                # X3 = E3 * f2
                if cc < 2:
                    # early chunks: the l3x exp may not have landed yet
                    # (its DMA chain is long); drain f2 to SBUF so ps2 slots
                    # recycle immediately, and multiply later (see cc==2)
                    nc.vector.tensor_copy(
                        f2e[32 * cc:32 * cc + 32, pc:pc + w], ps2[:])
                else:
                    # fused drain of ps2 (in place into l3x rows)
                    nc.vector.tensor_mul(
                        l3x[32 * cc:32 * cc + 32, pc:pc + w],
                        l3x[32 * cc:32 * cc + 32, pc:pc + w], ps2[:])

            def st_ps3(pc, w=PW):
                ps3 = ppool.tile([32, PW], F32, tag="psA", bufs=4,
                                 name="ps3")[:, 0:w]
                for s in range((w + MMN - 1) // MMN):
                    scol = s * MMN
                    sw = min(MMN, w - scol)
                    nc.tensor.matmul(
                        ps3[:, scol:scol + sw], g3t[:],
                        l3x[:, pc + scol:pc + scol + sw],
                        start=True, stop=True)
                return ps3

            def st_x4(pc, ps3, w=PW):
                nc.vector.tensor_mul(l4x[:, pc:pc + w], l4x[:, pc:pc + w],
                                     ps3[0:20, :])
                ps4 = ppool.tile([NCH, PW], F32, tag="psB", bufs=4,
                                 name="ps4")[:, 0:w]
                for s in range((w + MMN - 1) // MMN):
                    scol = s * MMN
                    sw = min(MMN, w - scol)
                    nc.tensor.matmul(
                        ps4[:, scol:scol + sw], g4t[:],
                        l4x[:, pc + scol:pc + scol + sw],
                        start=True, stop=True)
                return ps4

            def st_out(pc, ps4, tail, w=PW):
                if tail:
                    nc.scalar.copy(outsb[:, pc:pc + w], ps4[:])  # ACT idle
                else:
                    nc.vector.tensor_copy(outsb[:, pc:pc + w], ps4[:])

            def phase12(cc, pc, e1s):
                """levels 1+2 for chunk cc, columns [pc, pc+PW)."""
                ps1 = st_ps1(cc, pc, e1s)
                ps2 = st_x2(cc, pc, ps1)
                st_x3(cc, pc, ps2)

            def tail_pipeline(pcs, e1s, cc):
                """Chunk-3 pieces (pc, w) pairs, DVE ops emitted in skewed
                wavefronts so the strict-FIFO vector queue never stalls at
                its head."""
                n = len(pcs)
                ps1s = [st_ps1(cc, pc, e1s, w) for pc, w in pcs]
                ps2 = {}
                ps3 = {}
                ps4 = {}
                for d in range(n + 3):
                    for i, (pc, w) in reversed(list(enumerate(pcs))):
                        s = d - i
                        if s == 0:
                            ps2[i] = st_x2(cc, pc, ps1s[i], w)
                        elif s == 1:
                            st_x3(cc, pc, ps2[i], w)
                            ps3[i] = st_ps3(pc, w)
                        elif s == 2:
                            ps4[i] = st_x4(pc, ps3[i], w)
                        elif s == 3:
                            st_out(pc, ps4[i], tail=(pc >= 3072), w=w)

            outsb = bpool.tile([NCH, CHUNK], F32, tag="outsb", bufs=1)

            # ---------- per-chunk: exps then levels 1+2 ----------
            # chunks 0-2: whole-chunk exp calls; chunk 3: quarter-chunk calls
            # with phase 3/4 pieces interleaved so the post-exp tail is short
            NP = CHUNK // PW
            for cc in range(NCH):
                if cc == 2:
                    # deferred l3 mul for chunks 0/1 (fast fp16 SBUF mul)
                    nc.vector.tensor_mul(l3x[0:64, :], l3x[0:64, :],
                                         f2e[:, :])
                last = cc == NCH - 1
                e1s = [None] * NT1
                splits = [(q * 1024, 1024) for q in range(4)] if last \
                    else [(0, CHUNK)]
                for hh, (hc, w) in enumerate(splits):
                    for t in range(NT1):
                        if hh == 0:
                            e1s[t] = bpool.tile([128, CHUNK], F16, tag="e1",
                                                bufs=8, name=f"e1_{cc}_{t}")
                        nc.scalar.activation(
                            e1s[t][:, hc:hc + w], lreps[cc][:, hc:hc + w],
                            EXP, scale=sct1[:, t:t + 1])
                    if cc == 0 and hh == 0:
                        # hoist the small phase-B exps right behind e1 chunk 0
                        nc.scalar.activation(l3x[:], l3x[:], EXP,
                                             scale=sct3[:, 0:1])
                        nc.scalar.activation(e4pk[:], e4pk[:], EXP,
                                             scale=sct4[:, 0:1])
                        # unpack [80,1024] -> [20,4096] via DRAM bounce
                        # (partition-split SBUF source APs are unsupported)
                        nc.sync.dma_start(out=e4d[:, :], in_=e4pk[:, :])
                        nc.sync.dma_start(
                            out=l4x[:, :],
                            in_=e4d[:, :].rearrange("(m b) j -> m b j", b=4))
                    if hh == 0:
                        e2 = bpool.tile([128, CHUNK], F16, tag="e2", bufs=3,
                                        name=f"e2_{cc}")
                        e2s[cc] = e2
                    nc.scalar.activation(e2s[cc][:, hc:hc + w],
                                         lreps[cc][:, hc:hc + w], EXP,
                                         scale=sct2[:, 0:1])
                    if last:
                        pieces = [(p * PW, PW) for p in
                                  range(hc // PW, (hc + w) // PW)]
                        tail_pipeline(pieces, e1s, cc)
                    else:
                        for p in range(hc // PW, (hc + w) // PW):
                            phase12(cc, p * PW, e1s)

            nc.sync.dma_start(
                out=y[:].rearrange("(c i) -> c i", i=CHUNK), in_=outsb[:])

    nc.compile()
    return nc


def kernel(x, lam0, lam1, pow1, lam2, pow2, lam3, pow3, lam4, pow4):
    x = np.asarray(x, np.float32)
    consts = build_constants(
        np.asarray(lam0, np.float32), np.asarray(lam1, np.float32),
        np.asarray(pow1, np.float32), np.asarray(lam2, np.float32),
        np.asarray(pow2, np.float32), np.asarray(lam3, np.float32),
        np.asarray(pow3, np.float32), np.asarray(lam4, np.float32),
        np.asarray(pow4, np.float32))

    nc = build_bass()

    in_maps = []
    for k in range(M_CORES):
        shard = x[k * BS:(k + 1) * BS, :]
        m = {"xt": np.ascontiguousarray(shard.T)}
        m.update(consts)
        in_maps.append(m)

    from concourse.bass_utils import run_bass_kernel_spmd
    res = run_bass_kernel_spmd(nc, in_maps, list(range(M_CORES)))
    out = np.concatenate([res.results[k]["y"] for k in range(M_CORES)])
    return out[:, None].astype(np.float32)


if __name__ == "__main__":
    import reference
    inputs = {k: np.asarray(v) for k, v in reference.setup_inputs().items()}
    got = kernel(**inputs)
    exp = np.asarray(reference.reference(**inputs))
    err = np.abs(got - exp).max() / (np.abs(exp).max() + 1e-30)
    print("shape", got.shape, "relerr", err)
